# revision 36
# baseline (speedup 1.0000x reference)
"""Distributed ImprovedDilatedAttention on 8 Trainium2 NeuronCores.

Problem: [2, 4096, 12, 64] q/k/v, 3 head groups with (segment, dilation) in
[(1024,1), (2048,2), (4096,4)]. Each (group, batch, segment, head) pair is an
independent dense 1024x1024 attention over head_dim 64 (m = g/r = 1024 for
every group): 56 problems total, 7 per core.

Host side packs one bf16 input block per problem, [128, 2568] = qT | kT | vp:
  qT [128, 1024] = Q^T duplicated into both partition halves (row tiling)
  kT [128, 1024] = K^T duplicated likewise (stationary operand for S^T)
  vp [128, 8, 65] = V' chunks, V' = [V | 1]; vp[j, c, :] = V'[c*128 + j]
Device computes, per problem:
  S^T[kj, qi] = sum_d K^T[d,kj] Q^T[d,qi]   (4 chunks of 2 kj-blocks; the two
      blocks of a chunk run concurrently on PE row strips 0-63 / 64-127)
  E = exp(S^T / 8)                          (ACT, fp32 PSUM -> bf16 SBUF,
      one [128, 2048] ACTIVATE per chunk)
  out[qi, m] = sum_kj E[kj, qi] V'[kj, m]   (PV "quarters": 2 qi-blocks fully
      accumulated in PSUM, stationary = 128x128 E slices, moving = V')
PSUM: two single-buffered 4-bank pools; S chunks ping-pong between them so
ACT is never starved, and PV quarters borrow the just-freed slot. PV of
problem p is emitted interleaved with the S chunks of problem p+1.
out[:, 0:64] is the unnormalized O, col 64 is sumexp. Host divides and
scatters into the dilated positions (zeros elsewhere).
"""

import numpy as np

B, N, H, D = 2, 4096, 12, 64
SEG = [1024, 2048, 4096]
DIL = [1, 2, 4]
NGROUPS = 3
HPG = H // NGROUPS  # 4 heads per group
M = 1024            # dilated tokens per segment (g // r, same for all groups)
NPROB = 56
NCORES = 8
PPC = NPROB // NCORES  # 7 problems per core

_CACHE = {}
ROW_TILING = True


def _bf16():
    import ml_dtypes

    return ml_dtypes.bfloat16


def _groups():
    for i, (g, r) in enumerate(zip(SEG, DIL)):
        yield i, g, r, i % r, N // g


def _pack(query, key, value):
    """-> qT [56,64,1024], kT [56,64,1024] , vp [56,128,8,65] (all bf16)."""
    bf16 = _bf16()
    qs, ks, vs = [], [], []
    for i, g, r, off, s in _groups():
        idx = off + r * np.arange(g // r)
        hsl = slice(i * HPG, (i + 1) * HPG)

        def grab(x):
            return x.reshape(B, s, g, H, D)[:, :, idx][:, :, :, hsl, :]

        qg = grab(query)  # [B, s, m, hpg, D]
        kg = grab(key)
        vg = grab(value)
        qT = np.ascontiguousarray(qg.transpose(0, 1, 3, 4, 2)).reshape(-1, D, M)
        kT = np.ascontiguousarray(kg.transpose(0, 1, 3, 4, 2)).reshape(-1, D, M)
        # duplicate into both partition halves for 2-way PE row tiling
        qs.append(np.concatenate([qT, qT], axis=1))  # [n, 128, M]
        ks.append(np.concatenate([kT, kT], axis=1))
        v65 = np.concatenate(
            [vg, np.ones((*vg.shape[:-1], 1), np.float32)], axis=-1
        )  # [B, s, m, hpg, 65]
        vp = np.ascontiguousarray(v65.transpose(0, 1, 3, 2, 4)).reshape(-1, M, 65)
        vp = np.ascontiguousarray(vp.reshape(-1, 8, 128, 65).transpose(0, 2, 1, 3))
        vs.append(vp)
    qTp = np.concatenate(qs).astype(bf16)   # [56, 128, 1024]
    kTp = np.concatenate(ks).astype(bf16)   # [56, 128, 1024]
    vpp = np.concatenate(vs).astype(bf16)   # [56, 128, 8, 65]
    # one contiguous per-problem input block: [128, 1024 | 1024 | 520]
    return np.concatenate(
        [qTp, kTp, vpp.reshape(NPROB, 128, 520)], axis=2
    )  # [56, 128, 2568]


def _unpack(outT):
    """outT [56, 128, 8, 65] (j, c, m; qi = c*128 + j) -> full output."""
    o = outT.transpose(0, 2, 1, 3).reshape(NPROB, M, 65)  # [56, qi, 65]
    o = o[:, :, :64] / o[:, :, 64:65]  # [56, qi, 64]
    out = np.zeros((B, N, H, D), np.float32)
    ofs = 0
    for i, g, r, off, s in _groups():
        idx = off + r * np.arange(g // r)
        n_i = B * s * HPG
        og = o[ofs : ofs + n_i].reshape(B, s, HPG, M, D).transpose(0, 1, 3, 2, 4)
        out.reshape(B, s, g, H, D)[:, :, idx, i * HPG : (i + 1) * HPG, :] = og
        ofs += n_i
    return out


def _build(for_hw=True):
    import concourse.bacc as bacc
    import concourse.bass as bass
    import concourse.mybir as mybir
    import concourse.tile as tile

    f32 = mybir.dt.float32
    bf = mybir.dt.bfloat16
    nc = bacc.Bacc("TRN2", target_bir_lowering=False, debug=False,
                   enable_asserts=False)
    inx = nc.dram_tensor("inx", [PPC, 128, 2568], bf, kind="ExternalInput").ap()
    outT = nc.dram_tensor("outT", [PPC, 128, 8, 65], f32, kind="ExternalOutput").ap()

    with tile.TileContext(nc) as tc:
        with (
            tc.tile_pool(name="inp", bufs=4) as inp,
            tc.tile_pool(name="exps", bufs=4) as exps,
            tc.tile_pool(name="outp", bufs=4) as outp,
            tc.tile_pool(name="psumA", bufs=1, space=bass.MemorySpace.PSUM) as sA,
            tc.tile_pool(name="psumB", bufs=1, space=bass.MemorySpace.PSUM) as sB,
        ):
            # PSUM: two single-buffered [128, 2048]-f32 pools (4 banks each).
            # S chunks ping-pong sA/sB so ACT always has the next chunk
            # queued; PV "quarters" (2 qi-blocks fully accumulated, 16 MMs
            # ~0.9us) borrow the just-freed chunk slot and fit inside the
            # PE's natural wait for the next slot, so ACT never starves.

            def emit_s_chunk(qt, kt, eS, t, pool):
                # kj blocks j0 = 2t (rows 0-63) and j1 = 2t+1 (rows 64-127),
                # concurrent via PE row tiling; one wide exp drains both.
                j0, j1 = 2 * t, 2 * t + 1
                sch = pool.tile([128, 2, M], f32, tag="s")
                for c in range(2):  # qi chunks of 512
                    cs = slice(c * 512, (c + 1) * 512)
                    if ROW_TILING:
                        nc.tensor.matmul(
                            sch[:, 0, cs],
                            kt[0:64, j0 * 128 : (j0 + 1) * 128],
                            qt[0:64, cs],
                            start=True, stop=True,
                            tile_position=(0, 0),
                        )
                        nc.tensor.matmul(
                            sch[:, 1, cs],
                            kt[64:128, j1 * 128 : (j1 + 1) * 128],
                            qt[64:128, cs],
                            start=True, stop=True,
                            tile_position=(64, 0),
                        )
                    else:
                        nc.tensor.matmul(
                            sch[:, 0, cs],
                            kt[0:64, j0 * 128 : (j0 + 1) * 128],
                            qt[0:64, cs],
                            start=True, stop=True,
                        )
                        nc.tensor.matmul(
                            sch[:, 1, cs],
                            kt[0:64, j1 * 128 : (j1 + 1) * 128],
                            qt[0:64, cs],
                            start=True, stop=True,
                        )
                nc.scalar.activation(
                    eS[:, j0 : j0 + 2, :], sch,
                    mybir.ActivationFunctionType.Exp, scale=0.125,
                )

            def make_quarter(pprob, eSp, vptp, qpair):
                def emit(pool):
                    # qi blocks 2*qpair, 2*qpair+1 fully accumulated over
                    # all 8 kj chunks; block stride of 128 floats keeps each
                    # matmul output inside one PSUM bank
                    pvt = pool.tile([128, 2, 128], f32, tag="s")
                    for b in range(2):
                        qb = 2 * qpair + b
                        for c in range(8):
                            nc.tensor.matmul(
                                pvt[:, b, 0:65],
                                eSp[:, c, qb * 128 : (qb + 1) * 128],
                                vptp[:, c, :],
                                start=(c == 0),
                                stop=(c == 7),
                            )
                    ot = outp.tile([128, 2, 65], f32, tag="ot")
                    nc.vector.tensor_copy(out=ot, in_=pvt[:, :, 0:65])
                    # gpsimd (SWDGE) ring: keeps the sync HWDGE ring free
                    # for the input DMAs the ACT stream depends on
                    nc.gpsimd.dma_start(
                        out=outT[pprob][:, 2 * qpair : 2 * qpair + 2, :],
                        in_=ot,
                    )
                return emit

            from collections import deque

            pend = deque()
            for p in range(PPC):
                it = inp.tile([128, 2568], bf, tag="it")
                nc.sync.dma_start(out=it[:, 0:1284], in_=inx[p][:, 0:1284])
                nc.sync.dma_start(out=it[:, 1284:2568], in_=inx[p][:, 1284:2568])
                qt = it[:, 0:1024]
                kt = it[:, 1024:2048]
                vpt = it[:, 2048:2568].rearrange("p (c m) -> p c m", m=65)

                eS = exps.tile([128, 8, M], bf, tag="eS")
                for t in range(4):
                    pool = sA if t % 2 == 0 else sB
                    emit_s_chunk(qt, kt, eS, t, pool)
                    if pend:
                        pend.popleft()(pool)
                for i in range(4):
                    pend.append(make_quarter(p, eS, vpt, i))

            i = 0
            while pend:
                pend.popleft()(sA if i % 2 == 0 else sB)
                i += 1

    nc.compile()
    if for_hw:
        from concourse.bass_interp import get_hw_module

        nc.m = get_hw_module(nc.m)
    return nc


def _numpy_fallback(query, key, value, causal):
    out = np.zeros((B, N, H, D), np.float32)
    for i, g, r, off, s in _groups():
        idx = off + r * np.arange(g // r)
        hsl = slice(i * HPG, (i + 1) * HPG)
        q = query.reshape(B, s, g, H, D)[:, :, idx][:, :, :, hsl, :]
        k = key.reshape(B, s, g, H, D)[:, :, idx][:, :, :, hsl, :]
        v = value.reshape(B, s, g, H, D)[:, :, idx][:, :, :, hsl, :]
        scores = np.einsum("bsqhd,bskhd->bshqk", q, k) / np.sqrt(D).astype(np.float32)
        if causal:
            mask = np.tril(np.ones((g // r, g // r), dtype=bool))
            scores = np.where(mask, scores, np.float32(np.finfo(np.float32).min))
        scores -= scores.max(axis=-1, keepdims=True)
        p = np.exp(scores)
        p /= p.sum(axis=-1, keepdims=True)
        o = np.einsum("bshqk,bskhd->bsqhd", p, v)
        out.reshape(B, s, g, H, D)[:, :, idx, hsl, :] = o
    return out


def _in_maps(query, key, value):
    inx = _pack(query, key, value)
    return [
        {"inx": np.ascontiguousarray(inx[k * PPC : (k + 1) * PPC])}
        for k in range(NCORES)
    ]


def kernel(query, key, value, is_causal):
    query = np.asarray(query, dtype=np.float32)
    key = np.asarray(key, dtype=np.float32)
    value = np.asarray(value, dtype=np.float32)
    causal = bool(np.asarray(is_causal).item()) if np.ndim(is_causal) == 0 else bool(
        is_causal
    )
    if causal:
        return _numpy_fallback(query, key, value, causal)

    from concourse import bass_utils

    if "nc" not in _CACHE:
        _CACHE["nc"] = _build()
    nc = _CACHE["nc"]

    res = bass_utils.run_bass_kernel_spmd(
        nc, _in_maps(query, key, value), core_ids=list(range(NCORES))
    )
    outT = np.concatenate([res.results[k]["outT"] for k in range(NCORES)])
    return _unpack(outT)


# revision 38
# speedup vs baseline: 1.0297x; 1.0297x over previous
"""Distributed ImprovedDilatedAttention on 8 Trainium2 NeuronCores.

Problem: [2, 4096, 12, 64] q/k/v, 3 head groups with (segment, dilation) in
[(1024,1), (2048,2), (4096,4)]. Each (group, batch, segment, head) pair is an
independent dense 1024x1024 attention over head_dim 64 (m = g/r = 1024 for
every group): 56 problems total, 7 per core.

Host side packs one bf16 input block per problem, [128, 2568] = qT | kT | vp:
  qT [128, 1024] = Q^T duplicated into both partition halves (row tiling)
  kT [128, 1024] = K^T duplicated likewise (stationary operand for S^T)
  vp [128, 8, 65] = V' chunks, V' = [V | 1]; vp[j, c, :] = V'[c*128 + j]
Device computes, per problem:
  S^T[kj, qi] = sum_d K^T[d,kj] Q^T[d,qi]   (4 chunks of 2 kj-blocks; the two
      blocks of a chunk run concurrently on PE row strips 0-63 / 64-127)
  E = exp(S^T / 8)                          (ACT, fp32 PSUM -> bf16 SBUF,
      one [128, 2048] ACTIVATE per chunk)
  out[qi, m] = sum_kj E[kj, qi] V'[kj, m]   (PV "quarters": 2 qi-blocks fully
      accumulated in PSUM, stationary = 128x128 E slices, moving = V')
PSUM: two single-buffered 4-bank pools; S chunks ping-pong between them so
ACT is never starved, and PV quarters borrow the just-freed slot. PV of
problem p is emitted interleaved with the S chunks of problem p+1.
out[:, 0:64] is the unnormalized O, col 64 is sumexp. Host divides and
scatters into the dilated positions (zeros elsewhere).
"""

import numpy as np

B, N, H, D = 2, 4096, 12, 64
SEG = [1024, 2048, 4096]
DIL = [1, 2, 4]
NGROUPS = 3
HPG = H // NGROUPS  # 4 heads per group
M = 1024            # dilated tokens per segment (g // r, same for all groups)
NPROB = 56
NCORES = 8
PPC = NPROB // NCORES  # 7 problems per core

_CACHE = {}
ROW_TILING = True


def _bf16():
    import ml_dtypes

    return ml_dtypes.bfloat16


def _groups():
    for i, (g, r) in enumerate(zip(SEG, DIL)):
        yield i, g, r, i % r, N // g


def _pack(query, key, value):
    """-> qT [56,64,1024], kT [56,64,1024] , vp [56,128,8,65] (all bf16)."""
    bf16 = _bf16()
    qs, ks, vs = [], [], []
    for i, g, r, off, s in _groups():
        idx = off + r * np.arange(g // r)
        hsl = slice(i * HPG, (i + 1) * HPG)

        def grab(x):
            return x.reshape(B, s, g, H, D)[:, :, idx][:, :, :, hsl, :]

        qg = grab(query)  # [B, s, m, hpg, D]
        kg = grab(key)
        vg = grab(value)
        qT = np.ascontiguousarray(qg.transpose(0, 1, 3, 4, 2)).reshape(-1, D, M)
        kT = np.ascontiguousarray(kg.transpose(0, 1, 3, 4, 2)).reshape(-1, D, M)
        # duplicate into both partition halves for 2-way PE row tiling
        qs.append(np.concatenate([qT, qT], axis=1))  # [n, 128, M]
        ks.append(np.concatenate([kT, kT], axis=1))
        v65 = np.concatenate(
            [vg, np.ones((*vg.shape[:-1], 1), np.float32)], axis=-1
        )  # [B, s, m, hpg, 65]
        vp = np.ascontiguousarray(v65.transpose(0, 1, 3, 2, 4)).reshape(-1, M, 65)
        vp = np.ascontiguousarray(vp.reshape(-1, 8, 128, 65).transpose(0, 2, 1, 3))
        vs.append(vp)
    qTp = np.concatenate(qs).astype(bf16)   # [56, 128, 1024]
    kTp = np.concatenate(ks).astype(bf16)   # [56, 128, 1024]
    vpp = np.concatenate(vs).astype(bf16)   # [56, 128, 8, 65]
    # one contiguous per-problem input block: [128, 1024 | 1024 | 520]
    return np.concatenate(
        [qTp, kTp, vpp.reshape(NPROB, 128, 520)], axis=2
    )  # [56, 128, 2568]


def _unpack(outT):
    """outT [56, 128, 8, 65] (j, c, m; qi = c*128 + j) -> full output."""
    o = outT.transpose(0, 2, 1, 3).reshape(NPROB, M, 65)  # [56, qi, 65]
    o = o[:, :, :64] / o[:, :, 64:65]  # [56, qi, 64]
    out = np.zeros((B, N, H, D), np.float32)
    ofs = 0
    for i, g, r, off, s in _groups():
        idx = off + r * np.arange(g // r)
        n_i = B * s * HPG
        og = o[ofs : ofs + n_i].reshape(B, s, HPG, M, D).transpose(0, 1, 3, 2, 4)
        out.reshape(B, s, g, H, D)[:, :, idx, i * HPG : (i + 1) * HPG, :] = og
        ofs += n_i
    return out


def _build(for_hw=True):
    import concourse.bacc as bacc
    import concourse.bass as bass
    import concourse.mybir as mybir
    import concourse.tile as tile

    f32 = mybir.dt.float32
    bf = mybir.dt.bfloat16
    nc = bacc.Bacc("TRN2", target_bir_lowering=False, debug=False,
                   enable_asserts=False)
    inx = nc.dram_tensor("inx", [PPC, 128, 2568], bf, kind="ExternalInput").ap()
    outT = nc.dram_tensor("outT", [PPC, 128, 8, 65], f32, kind="ExternalOutput").ap()

    with tile.TileContext(nc) as tc:
        with (
            tc.tile_pool(name="inp", bufs=4) as inp,
            tc.tile_pool(name="exps", bufs=4) as exps,
            tc.tile_pool(name="outp", bufs=4) as outp,
            tc.tile_pool(name="psumA", bufs=1, space=bass.MemorySpace.PSUM) as sA,
            tc.tile_pool(name="psumB", bufs=1, space=bass.MemorySpace.PSUM) as sB,
        ):
            # PSUM: two single-buffered [128, 2048]-f32 pools (4 banks each).
            # S chunks ping-pong sA/sB so ACT always has the next chunk
            # queued; PV "quarters" (2 qi-blocks fully accumulated, 16 MMs
            # ~0.9us) borrow the just-freed chunk slot and fit inside the
            # PE's natural wait for the next slot, so ACT never starves.

            def emit_s_chunk(qt, kt, eS, t, pool):
                # kj blocks j0 = 2t (rows 0-63) and j1 = 2t+1 (rows 64-127),
                # concurrent via PE row tiling; one wide exp drains both.
                j0, j1 = 2 * t, 2 * t + 1
                sch = pool.tile([128, 2, M], f32, tag="s")
                for c in range(2):  # qi chunks of 512
                    cs = slice(c * 512, (c + 1) * 512)
                    if ROW_TILING:
                        nc.tensor.matmul(
                            sch[:, 0, cs],
                            kt[0:64, j0 * 128 : (j0 + 1) * 128],
                            qt[0:64, cs],
                            start=True, stop=True,
                            tile_position=(0, 0),
                        )
                        nc.tensor.matmul(
                            sch[:, 1, cs],
                            kt[64:128, j1 * 128 : (j1 + 1) * 128],
                            qt[64:128, cs],
                            start=True, stop=True,
                            tile_position=(64, 0),
                        )
                    else:
                        nc.tensor.matmul(
                            sch[:, 0, cs],
                            kt[0:64, j0 * 128 : (j0 + 1) * 128],
                            qt[0:64, cs],
                            start=True, stop=True,
                        )
                        nc.tensor.matmul(
                            sch[:, 1, cs],
                            kt[0:64, j1 * 128 : (j1 + 1) * 128],
                            qt[0:64, cs],
                            start=True, stop=True,
                        )
                nc.scalar.activation(
                    eS[:, j0 : j0 + 2, :], sch,
                    mybir.ActivationFunctionType.Exp, scale=0.125,
                )

            def make_quarter(pprob, eSp, vptp, qpair):
                def emit(pool):
                    # qi blocks 2*qpair, 2*qpair+1 fully accumulated over
                    # all 8 kj chunks; block stride of 128 floats keeps each
                    # matmul output inside one PSUM bank
                    pvt = pool.tile([128, 2, 128], f32, tag="s")
                    for b in range(2):
                        qb = 2 * qpair + b
                        for c in range(8):
                            nc.tensor.matmul(
                                pvt[:, b, 0:65],
                                eSp[:, c, qb * 128 : (qb + 1) * 128],
                                vptp[:, c, :],
                                start=(c == 0),
                                stop=(c == 7),
                            )
                    ot = outp.tile([128, 2, 65], f32, tag="ot")
                    nc.vector.tensor_copy(out=ot, in_=pvt[:, :, 0:65])
                    nc.sync.dma_start(
                        out=outT[pprob][:, 2 * qpair : 2 * qpair + 2, :],
                        in_=ot,
                    )
                return emit

            from collections import deque

            pend = deque()
            for p in range(PPC):
                it = inp.tile([128, 2568], bf, tag="it")
                # first piece = Q + first-half K: S chunks t0/t1 depend only
                # on it, so the PE can start one DMA earlier
                nc.sync.dma_start(out=it[:, 0:1536], in_=inx[p][:, 0:1536])
                nc.sync.dma_start(out=it[:, 1536:2568], in_=inx[p][:, 1536:2568])
                qt = it[:, 0:1024]
                kt = it[:, 1024:2048]
                vpt = it[:, 2048:2568].rearrange("p (c m) -> p c m", m=65)

                eS = exps.tile([128, 8, M], bf, tag="eS")
                for t in range(4):
                    pool = sA if t % 2 == 0 else sB
                    emit_s_chunk(qt, kt, eS, t, pool)
                    if pend:
                        pend.popleft()(pool)
                for i in range(4):
                    pend.append(make_quarter(p, eS, vpt, i))

            i = 0
            while pend:
                pend.popleft()(sA if i % 2 == 0 else sB)
                i += 1

    nc.compile()
    if for_hw:
        from concourse.bass_interp import get_hw_module

        nc.m = get_hw_module(nc.m)
    return nc


def _numpy_fallback(query, key, value, causal):
    out = np.zeros((B, N, H, D), np.float32)
    for i, g, r, off, s in _groups():
        idx = off + r * np.arange(g // r)
        hsl = slice(i * HPG, (i + 1) * HPG)
        q = query.reshape(B, s, g, H, D)[:, :, idx][:, :, :, hsl, :]
        k = key.reshape(B, s, g, H, D)[:, :, idx][:, :, :, hsl, :]
        v = value.reshape(B, s, g, H, D)[:, :, idx][:, :, :, hsl, :]
        scores = np.einsum("bsqhd,bskhd->bshqk", q, k) / np.sqrt(D).astype(np.float32)
        if causal:
            mask = np.tril(np.ones((g // r, g // r), dtype=bool))
            scores = np.where(mask, scores, np.float32(np.finfo(np.float32).min))
        scores -= scores.max(axis=-1, keepdims=True)
        p = np.exp(scores)
        p /= p.sum(axis=-1, keepdims=True)
        o = np.einsum("bshqk,bskhd->bsqhd", p, v)
        out.reshape(B, s, g, H, D)[:, :, idx, hsl, :] = o
    return out


def _in_maps(query, key, value):
    inx = _pack(query, key, value)
    return [
        {"inx": np.ascontiguousarray(inx[k * PPC : (k + 1) * PPC])}
        for k in range(NCORES)
    ]


def kernel(query, key, value, is_causal):
    query = np.asarray(query, dtype=np.float32)
    key = np.asarray(key, dtype=np.float32)
    value = np.asarray(value, dtype=np.float32)
    causal = bool(np.asarray(is_causal).item()) if np.ndim(is_causal) == 0 else bool(
        is_causal
    )
    if causal:
        return _numpy_fallback(query, key, value, causal)

    from concourse import bass_utils

    if "nc" not in _CACHE:
        _CACHE["nc"] = _build()
    nc = _CACHE["nc"]

    res = bass_utils.run_bass_kernel_spmd(
        nc, _in_maps(query, key, value), core_ids=list(range(NCORES))
    )
    outT = np.concatenate([res.results[k]["outT"] for k in range(NCORES)])
    return _unpack(outT)
